# revision 13
# baseline (speedup 1.0000x reference)
"""Trainium2 Bass kernel for nn_AttentionBlock (B=16, C=256, H=W=32, 4 heads, d=64).

Strategy: data-parallel over batch across 8 cores (2 batches/core), no
collectives.  Per core the attention block is computed entirely in a
"transposed" layout so no on-device transposes are needed:

  x[b]           : [C, S]   (natural DRAM layout, S = H*W = 1024)
  qkT            : [128, S] tiles (pair-of-heads packed: q_h0|q_h1, k_h0|k_h1)
  scoresT[j, i]  : via matmul(lhsT=kT, rhs=qT), 2 heads concurrently in
                   disjoint PE row groups (K=64 each)
  PT = exp(scoresT) : ACT, stays in SBUF (softmax max-subtraction is skipped:
                   scores are ~N(0,1) after the 1/8 scale, exp can't overflow)
  outT[d, i]     : via matmul(lhsT=v[j,d], rhs=PT); a concurrent M=1 all-ones
                   matmul in a disjoint PE column group produces the softmax
                   row-sums essentially for free
  normalization  : reciprocal(rowsum) -> DMA partition-broadcast -> DVE mul
  res^T[c, s]    : output projection (+bias as K=1 rank-1 matmul) + fp32
                   residual add

Host-side folds: q-scale (1/8) into w_proj/b_proj q columns; v-bias into the
output bias (softmax rows sum to 1); bias adds become K=1 matmuls.
Matmul operands are bf16 (1 PE cycle/row vs 4 for fp32); accumulation and the
softmax/normalization/residual arithmetic stay fp32.
"""

import numpy as np
import ml_dtypes
from contextlib import ExitStack

import concourse.bass as bass
import concourse.tile as tile
from concourse import bacc, mybir
from concourse.bass_utils import run_bass_kernel_spmd

F32 = mybir.dt.float32
BF16 = mybir.dt.bfloat16
NPBF = ml_dtypes.bfloat16

N_CORES = 8
B, C, H, W = 16, 256, 32, 32
S = H * W                     # 1024
N_HEADS, D_K = 4, 64
SCALE = D_K ** -0.5
B_LOC = B // N_CORES          # 2 batches per core

_NC_CACHE: dict = {}


def build_nc(repeat: int = 1):
    """Build + compile the per-core Bass program. SPMD: same program on all
    cores, different data."""
    if repeat in _NC_CACHE:
        return _NC_CACHE[repeat]

    nc = bacc.Bacc("TRN2", target_bir_lowering=False, debug=False)

    x16_d = nc.dram_tensor("x16", [B_LOC, C, S], BF16, kind="ExternalInput").ap()
    x32_d = nc.dram_tensor("x32", [B_LOC, C, S], F32, kind="ExternalInput").ap()
    wqk_d = nc.dram_tensor("w_qk", [C, 512], BF16, kind="ExternalInput").ap()
    wv_d = nc.dram_tensor("w_v", [C, 256], BF16, kind="ExternalInput").ap()
    wo_d = nc.dram_tensor("w_out", [C, C], BF16, kind="ExternalInput").ap()
    bqk_d = nc.dram_tensor("b_qk", [512], BF16, kind="ExternalInput").ap()
    beff_d = nc.dram_tensor("b_eff", [C], BF16, kind="ExternalInput").ap()
    out_d = nc.dram_tensor("out", [B_LOC, C, S], F32, kind="ExternalOutput").ap()

    with tile.TileContext(nc) as tc, ExitStack() as ctx:
        consts = ctx.enter_context(tc.tile_pool(name="consts", bufs=1))
        xpool = ctx.enter_context(tc.tile_pool(name="xp", bufs=2))
        x32pool = ctx.enter_context(tc.tile_pool(name="x32p", bufs=2))
        qkpool = ctx.enter_context(tc.tile_pool(name="qk", bufs=5))
        vpool = ctx.enter_context(tc.tile_pool(name="vp", bufs=2))
        ptpool = ctx.enter_context(tc.tile_pool(name="pt", bufs=18))
        avpool = ctx.enter_context(tc.tile_pool(name="av", bufs=3))
        rspool = ctx.enter_context(tc.tile_pool(name="rs", bufs=3))
        rbpool = ctx.enter_context(tc.tile_pool(name="rb", bufs=3))
        otpool = ctx.enter_context(tc.tile_pool(name="ot", bufs=2))
        respool = ctx.enter_context(tc.tile_pool(name="res", bufs=3))
        drpool = ctx.enter_context(tc.tile_pool(name="dr", bufs=4, space="DRAM"))
        psum = ctx.enter_context(tc.tile_pool(name="ps", bufs=1, space="PSUM"))

        # ---- constants ----
        wqk_sb = consts.tile([128, 2, 512], BF16)
        nc.sync.dma_start(wqk_sb[:], wqk_d.rearrange("(kt p) m -> p kt m", p=128))
        wv_sb = consts.tile([128, 2, 256], BF16)
        nc.sync.dma_start(wv_sb[:], wv_d.rearrange("(kt p) m -> p kt m", p=128))
        wo_sb = consts.tile([128, 4, 256], BF16)
        nc.sync.dma_start(
            wo_sb[64:128, :, :], wo_d.rearrange("(h p) m -> p h m", p=64)
        )
        bqk_sb = consts.tile([1, 512], BF16)
        nc.sync.dma_start(bqk_sb[:], bqk_d.unsqueeze(0))
        beff_sb = consts.tile([1, 256], BF16)
        nc.sync.dma_start(beff_sb[:], beff_d.unsqueeze(0))
        ones_x = consts.tile([1, 1024], BF16)
        nc.vector.memset(ones_x[:], 1.0)
        ones_col = consts.tile([128, 1], BF16)
        nc.vector.memset(ones_col[:], 1.0)

        def body(_iv=None):
            for b in range(B_LOC):
                x_t = xpool.tile([128, 2, 1024], BF16)
                nc.sync.dma_start(
                    x_t[:], x16_d[b].rearrange("(kt p) s -> p kt s", p=128)
                )
                x32_t = x32pool.tile([128, 2, 1024], F32)
                nc.sync.dma_start(
                    x32_t[:], x32_d[b].rearrange("(kt p) s -> p kt s", p=128)
                )

                # ---- qkT projection: 4 M-tiles [qp0, kp0, qp1, kp1] ----
                qk_tiles = []
                for mt in range(4):
                    ps = psum.tile([128, 1024], F32, tag="misc")
                    for ch in range(2):
                        cs = slice(ch * 512, (ch + 1) * 512)
                        for kt in range(2):
                            nc.tensor.matmul(
                                ps[:, cs],
                                lhsT=wqk_sb[:, kt, mt * 128:(mt + 1) * 128],
                                rhs=x_t[:, kt, cs],
                                start=(kt == 0), stop=False,
                            )
                        nc.tensor.matmul(  # + bias (rank-1: b ⊗ ones)
                            ps[:, cs],
                            lhsT=bqk_sb[:, mt * 128:(mt + 1) * 128],
                            rhs=ones_x[:, cs],
                            start=False, stop=True,
                        )
                    qk_t = qkpool.tile([128, 1024], BF16)
                    nc.vector.tensor_copy(qk_t[:], ps[:])
                    qk_tiles.append(qk_t)

                # ---- v: [j, 4*64] per j-tile ----
                v_t = vpool.tile([128, 8, 256], BF16)
                for jt in range(8):
                    psv = psum.tile([128, 256], F32, tag="misc")
                    for kt in range(2):
                        nc.tensor.matmul(
                            psv[:],
                            lhsT=x_t[:, kt, jt * 128:(jt + 1) * 128],
                            rhs=wv_sb[:, kt, :],
                            start=(kt == 0), stop=(kt == 1),
                        )
                    nc.vector.tensor_copy(v_t[:, jt, :], psv[:])

                outT = [
                    otpool.tile([128, 1024], BF16, tag=f"ot{i}", name=f"outT{i}")
                    for i in range(4)
                ]

                # ---- attention, one pair of heads at a time ----
                for p in range(2):
                    q_t, k_t = qk_tiles[2 * p], qk_tiles[2 * p + 1]
                    pts = []
                    for jt in range(8):
                        js = slice(jt * 128, (jt + 1) * 128)
                        psA = psum.tile([128, 1024], F32, tag="sA")
                        psB = psum.tile([128, 1024], F32, tag="sB")
                        for ch in range(2):
                            cs = slice(ch * 512, (ch + 1) * 512)
                            nc.tensor.matmul(  # head 2p in PE rows 0-63
                                psA[:, cs],
                                lhsT=k_t[0:64, js], rhs=q_t[0:64, cs],
                                start=True, stop=True, tile_position=(0, 0),
                            )
                            nc.tensor.matmul(  # head 2p+1 in PE rows 64-127
                                psB[:, cs],
                                lhsT=k_t[64:128, js], rhs=q_t[64:128, cs],
                                start=True, stop=True, tile_position=(64, 0),
                            )
                        ptA = ptpool.tile([128, 1024], BF16)
                        ptB = ptpool.tile([128, 1024], BF16)
                        nc.scalar.activation(
                            ptA[:], psA[:], mybir.ActivationFunctionType.Exp
                        )
                        nc.scalar.activation(
                            ptB[:], psB[:], mybir.ActivationFunctionType.Exp
                        )
                        pts.append((ptA, ptB))

                    for hh in (2 * p, 2 * p + 1):
                        odd = hh % 2
                        for ch in range(2):
                            cs = slice(ch * 512, (ch + 1) * 512)
                            ps_av = psum.tile([128, 512], F32, tag="av")
                            ps_rs = psum.tile([128, 512], F32, tag="rsum")
                            for jt in range(8):
                                pt = pts[jt][odd]
                                nc.tensor.matmul(  # rowsum += 1^T P_h^T
                                    ps_rs[0:1, :],
                                    lhsT=ones_col[:],
                                    rhs=pt[:, cs],
                                    start=(jt == 0), stop=(jt == 7),
                                    tile_position=(0, 0),
                                )
                                nc.tensor.matmul(  # out_h^T += v_h^T P_h^T
                                    ps_av[64:128, :],
                                    lhsT=v_t[:, jt, hh * 64:(hh + 1) * 64],
                                    rhs=pt[:, cs],
                                    start=(jt == 0), stop=(jt == 7),
                                    tile_position=(0, 64),
                                )
                            rs_t = rspool.tile([128, 512], F32)
                            nc.vector.reciprocal_approx_fast(
                                rs_t[0:1, :], ps_rs[0:1, :]
                            )
                            av_sb = avpool.tile([128, 512], F32)
                            nc.vector.tensor_copy(av_sb[64:128, :], ps_av[64:128, :])
                            rs_dr = drpool.tile([1, 512], F32)
                            nc.sync.dma_start(rs_dr[:], rs_t[0:1, :])
                            rb_t = rbpool.tile([128, 512], F32)
                            nc.sync.dma_start(  # broadcast 1/rowsum over d
                                rb_t[64:128, :],
                                bass.AP(
                                    tensor=rs_dr.tensor,
                                    offset=rs_dr.offset,
                                    ap=[[0, 64]] + rs_dr.ap[1:],
                                ),
                            )
                            nc.vector.tensor_mul(
                                outT[hh][64:128, cs],
                                av_sb[64:128, :], rb_t[64:128, :],
                            )

                # ---- output projection + residual ----
                for ct in range(2):
                    pso = psum.tile([128, 1024], F32, tag="misc")
                    for ch in range(2):
                        cs = slice(ch * 512, (ch + 1) * 512)
                        for hh in range(4):
                            nc.tensor.matmul(
                                pso[:, cs],
                                lhsT=wo_sb[64:128, hh, ct * 128:(ct + 1) * 128],
                                rhs=outT[hh][64:128, cs],
                                start=(hh == 0), stop=False,
                                tile_position=(64, 0),
                            )
                        nc.tensor.matmul(  # + b_eff (rank-1)
                            pso[:, cs],
                            lhsT=beff_sb[:, ct * 128:(ct + 1) * 128],
                            rhs=ones_x[:, cs],
                            start=False, stop=True,
                        )
                    res_t = respool.tile([128, 1024], F32)
                    nc.vector.tensor_add(res_t[:], pso[:], x32_t[:, ct, :])
                    nc.sync.dma_start(
                        out_d[b, ct * 128:(ct + 1) * 128, :], res_t[:]
                    )

        if repeat == 1:
            body()
        else:
            with tc.For_i(0, repeat, 1) as iv:
                body(iv)

    nc.compile()
    _NC_CACHE[repeat] = nc
    return nc


def host_prep(w_proj, b_proj, w_out, b_out):
    """Fold q-scale and v-bias; reorder columns into the device layout."""
    w_proj = np.asarray(w_proj, np.float32)
    b_proj = np.asarray(b_proj, np.float32)
    w_out = np.asarray(w_out, np.float32)
    b_out = np.asarray(b_out, np.float32)

    wq = [w_proj[:, h * 192:h * 192 + 64] * SCALE for h in range(N_HEADS)]
    wk = [w_proj[:, h * 192 + 64:h * 192 + 128] for h in range(N_HEADS)]
    wv = [w_proj[:, h * 192 + 128:h * 192 + 192] for h in range(N_HEADS)]
    bq = [b_proj[h * 192:h * 192 + 64] * SCALE for h in range(N_HEADS)]
    bk = [b_proj[h * 192 + 64:h * 192 + 128] for h in range(N_HEADS)]
    bv = [b_proj[h * 192 + 128:h * 192 + 192] for h in range(N_HEADS)]

    w_qk = np.ascontiguousarray(np.concatenate(
        [wq[0], wq[1], wk[0], wk[1], wq[2], wq[3], wk[2], wk[3]], axis=1
    ).astype(NPBF))
    b_qk = np.ascontiguousarray(np.concatenate(
        [bq[0], bq[1], bk[0], bk[1], bq[2], bq[3], bk[2], bk[3]]
    ).astype(NPBF))
    w_v = np.ascontiguousarray(np.concatenate(wv, axis=1).astype(NPBF))
    b_v = np.concatenate(bv)
    b_eff = (b_out + b_v @ w_out).astype(NPBF)
    w_o16 = np.ascontiguousarray(w_out.astype(NPBF))
    return w_qk, b_qk, w_v, w_o16, b_eff


def kernel(x, w_proj, b_proj, w_out, b_out):
    x = np.asarray(x, np.float32)
    w_qk, b_qk, w_v, w_o16, b_eff = host_prep(w_proj, b_proj, w_out, b_out)

    nc = build_nc(repeat=1)
    xs = x.reshape(B, C, S)
    in_maps = []
    for c in range(N_CORES):
        xc = np.ascontiguousarray(xs[c * B_LOC:(c + 1) * B_LOC])
        in_maps.append({
            "x16": xc.astype(NPBF), "x32": xc,
            "w_qk": w_qk, "w_v": w_v, "w_out": w_o16,
            "b_qk": b_qk, "b_eff": b_eff,
        })
    res = run_bass_kernel_spmd(nc, in_maps, list(range(N_CORES)))
    out = np.concatenate([res.results[c]["out"] for c in range(N_CORES)], axis=0)
    return out.reshape(B, C, H, W)
